# revision 20
# baseline (speedup 1.0000x reference)
"""Trainium2 Bass kernel for causal multi-head attention with RoPE.

Problem: B=4, S=2048, D=768, H=12, HD=64 (torch-Linear style projections,
rotary embeddings on q/k, causal softmax, output projection + bias).

Sharding across 8 NeuronCores: core c handles batch c//2 and head-group
c%2 (6 of 12 heads). Each core computes a partial output projection
(its heads' contribution to ctx @ Wo.T); the host sums the two partials
per batch and adds the bias. No device collectives.

Per-core kernel:
  - Projections run in bf16 (inputs/weights shipped as bf16; PE rate is
    identical to fp32r and DMA bytes halve). Q^T/K^T [hd, S] computed via
    pre-transposed weights; RoPE fused into PSUM eviction: cos-mult on
    DVE, the four rotate-half sin-mults on Pool (32-partition slices),
    final add on DVE. V [S, hd] with an appended ones column (Pool evict).
  - Attention in fp32r: per head, scores^T [k, q] = K_j^T.T @ Q^T (causal
    chunks only), exp on ScalarE with scale=1/8 folded in, PV with [V|1]
    stationary so PSUM row 64 is the softmax denominator for free; PV is
    emitted one k-chunk behind exp so PE's in-order queue never stalls.
  - The whole program is a fine-grained weave: all six heads' low halves
    (q < 1024) run first, with projection chunks and V blocks inserted
    between attention k-steps so PE always has independent work while
    ScalarE catches up on exp; high halves follow the same pattern, and
    the out-projection of rows 0-1023 is folded into the last head's
    attention stream. Normalization multiplies whole head-pairs
    ([128,1024] per op) with reciprocals DMA-broadcast across partitions.
"""

import numpy as np

B, S, D, H = 4, 2048, 768, 12
HD = D // H          # 64
N_CORES = 8
HEADS_PER_CORE = 6
PAIRS = 3            # head pairs per core
DC = D // 128        # 6 contraction chunks
MC = HEADS_PER_CORE * HD // 128   # 3 output-dim chunks (pairs)
NJ = S // 128        # 16 k-chunks
HW_ = 1024           # q-half width

_CACHE = {}


def _rope_tables():
    """cos/sin tables in the pair-interleaved hd layout.

    q/k features are stored with rotate-half partners adjacent (partition
    2f holds x1[f], partition 2f+1 holds x2[f], per head) so the rotate
    becomes a swap of adjacent lanes inside each 32-lane quadrant — a
    single DVE stream_shuffle.  sinM carries the rotate signs: row 2f
    multiplies the shuffled x2[f] by -sin, row 2f+1 multiplies x1[f] by
    +sin."""
    inv_freq = 1.0 / (10000.0 ** (np.arange(0, HD, 2, dtype=np.float64) / HD))
    ang = np.arange(S, dtype=np.float64)[:, None] * inv_freq[None, :]  # [S, 32]
    cos = np.cos(ang).astype(np.float32)   # [S, 32]
    sin = np.sin(ang).astype(np.float32)
    cosF = np.empty((128, S), np.float32)
    sinM = np.empty((128, S), np.float32)
    for h2 in range(2):
        for f in range(32):
            p = 64 * h2 + 2 * f
            cosF[p] = cosF[p + 1] = cos.T[f]
            sinM[p] = -sin.T[f]
            sinM[p + 1] = sin.T[f]
    return cosF, sinM


def _qk_perm():
    """Column permutation of Wq/Wk into the pair-interleaved layout."""
    perm = np.empty(384, np.int64)
    for h6 in range(HEADS_PER_CORE):
        base = 64 * h6
        for f in range(32):
            perm[base + 2 * f] = base + f
            perm[base + 2 * f + 1] = base + 32 + f
    return perm


def _build_program(reps=1):
    import concourse.bacc as bacc
    import concourse.mybir as mybir
    import concourse.tile as tile

    f32 = mybir.dt.float32
    f32r = mybir.dt.float32r
    bf16 = mybir.dt.bfloat16
    AF = mybir.ActivationFunctionType
    OP = mybir.AluOpType

    SWAP_MASK = [i ^ 1 for i in range(32)]   # swap adjacent lanes

    nc = bacc.Bacc("TRN2", target_bir_lowering=False, debug=False,
                   num_devices=N_CORES)

    eT = nc.declare_dram_parameter("eT", [D, S], bf16, isOutput=False)
    wq = nc.declare_dram_parameter("wq", [D, 384], bf16, isOutput=False)
    wk = nc.declare_dram_parameter("wk", [D, 384], bf16, isOutput=False)
    wv = nc.declare_dram_parameter("wv", [D, 384], bf16, isOutput=False)
    wo = nc.declare_dram_parameter("wo", [384, D], f32r, isOutput=False)
    cosF_d = nc.declare_dram_parameter("cosF", [128, S], bf16, isOutput=False)
    sinM_d = nc.declare_dram_parameter("sinM", [128, S], bf16, isOutput=False)
    mask_d = nc.declare_dram_parameter("mask", [128, 128], f32, isOutput=False)
    o = nc.declare_dram_parameter("o", [S, D], f32, isOutput=True)

    with tile.TileContext(nc) as tc, \
            nc.allow_low_precision(reason="bf16 projections / tf32 attention"):
        with tc.tile_pool(name="const", bufs=1) as cp:
            cosF = cp.tile([128, S], bf16)
            sinM = cp.tile([128, S], bf16)
            msk = cp.tile([128, 128], f32)

            qt = cp.tile([128, PAIRS, S], f32r)
            kt = cp.tile([128, PAIRS, S], f32r)
            vt = cp.tile([128, NJ, HEADS_PER_CORE, HD + 1], f32r)
            nc.vector.memset(vt[:, :, :, HD].bitcast(mybir.dt.uint32),
                             0x3F800000)
            wot = cp.tile([128, MC, D], f32r)
            rcs = cp.tile([128, 3, HW_], f32)

            eT_r = eT[:].rearrange("(n p) s -> p n s", p=128)
            wq_r = wq[:].rearrange("(n p) m -> p n m", p=128)
            wk_r = wk[:].rearrange("(n p) m -> p n m", p=128)
            wv_r = wv[:].rearrange("(n p) m -> p n m", p=128)
            wo_r = wo[:].rearrange("(n p) m -> p n m", p=128)

            for _rep in range(reps):
                with (
                    tc.tile_pool(name="pc", bufs=1) as pc,
                    tc.tile_pool(name="wts", bufs=1) as wts,
                    tc.tile_pool(name="wrk", bufs=1) as wrk,
                ):
                    cxt = pc.tile([128, PAIRS, S], f32r)   # unnormalized ctx^T

                    # ---- DMA prologue ----
                    # SP queue: the eT stream (all four 512-col chunks live
                    # at once; bf16 keeps the ring at 12 KB/partition).
                    ets = []
                    for cc in range(4):
                        cols = slice(512 * cc, 512 * cc + 512)
                        etA = wrk.tile([128, 3, 512], bf16, tag="et", bufs=8,
                                       name=f"eA{cc}")
                        nc.sync.dma_start(etA[:], eT_r[:, 0:3, cols])
                        etB = wrk.tile([128, 3, 512], bf16, tag="et", bufs=8,
                                       name=f"eB{cc}")
                        nc.sync.dma_start(etB[:], eT_r[:, 3:6, cols])
                        ets.append((etA, etB))

                    # Act queue: weights + tables, ordered by first use.
                    wqt = wts.tile([128, DC, 384], bf16)
                    for p in range(PAIRS):
                        cs = slice(128 * p, 128 * p + 128)
                        nc.scalar.dma_start(wqt[:, :, cs], wq_r[:, :, cs])
                    nc.scalar.dma_start(cosF[:, 0:512], cosF_d[:, 0:512])
                    nc.scalar.dma_start(sinM[:, 0:512], sinM_d[:, 0:512])
                    wkt = wts.tile([128, DC, 384], bf16)
                    nc.scalar.dma_start(wkt[:], wk_r)
                    nc.scalar.dma_start(msk[:], mask_d[:])
                    wvt = wts.tile([128, DC, 384], bf16)
                    nc.scalar.dma_start(wvt[:], wv_r)
                    nc.scalar.dma_start(cosF[:, 512:1024], cosF_d[:, 512:1024])
                    nc.scalar.dma_start(sinM[:, 512:1024], sinM_d[:, 512:1024])
                    nc.scalar.dma_start(wot[:], wo_r)
                    nc.scalar.dma_start(cosF[:, 1024:], cosF_d[:, 1024:])
                    nc.scalar.dma_start(sinM[:, 1024:], sinM_d[:, 1024:])

                    scp_cm = tc.tile_pool(name="scp", bufs=2, space="PSUM")
                    scp = scp_cm.__enter__()
                    cxp_cm = tc.tile_pool(name="cxp", bufs=1, space="PSUM")
                    cxp = cxp_cm.__enter__()
                    pps_cm = tc.tile_pool(name="pps", bufs=2, space="PSUM")
                    pps = pps_cm.__enter__()

                    def et(cc, d):
                        return ets[cc][0 if d < 3 else 1][:, d % 3, :]

                    def qk_unit(pair, wt, dst, cc):
                        """Q^T or K^T for one pair, one 512-col chunk, with
                        RoPE fused into the PSUM eviction."""
                        cols = slice(512 * cc, 512 * cc + 512)
                        tag = "q" if dst is qt else "k"
                        ps = pps.tile([128, 512], f32, tag="ps",
                                      name=f"ps{tag}{pair}{cc}")
                        for d in range(DC):
                            nc.tensor.matmul(
                                ps[:],
                                wt[:, d, 128 * pair:128 * pair + 128],
                                et(cc, d),
                                start=(d == 0), stop=(d == DC - 1))
                        psw = wrk.tile([128, 512], f32, tag="sh", bufs=2,
                                       name=f"w{tag}{pair}{cc}")
                        nc.vector.stream_shuffle(psw[:], ps[:], SWAP_MASK)
                        t_t = wrk.tile([128, 512], f32, tag="t", bufs=2,
                                       name=f"t{tag}{pair}{cc}")
                        nc.vector.tensor_tensor(
                            t_t[:], ps[:], cosF[:, cols], OP.mult)
                        u_t = wrk.tile([128, 512], f32, tag="u", bufs=2,
                                       name=f"u{tag}{pair}{cc}")
                        nc.vector.tensor_tensor(
                            u_t[:], psw[:], sinM[:, cols], OP.mult)
                        nc.vector.tensor_tensor(
                            dst[:, pair, cols], t_t[:], u_t[:], OP.add)

                    def pv_unit(i):
                        """V rows [128i, 128i+128) for all heads."""
                        cc, io = i // 4, 128 * (i % 4)
                        pvp = pps.tile([128, 384], f32, tag="ps",
                                       name=f"pv{i}")
                        for d in range(DC):
                            nc.tensor.matmul(
                                pvp[:], et(cc, d)[:, io:io + 128],
                                wvt[:, d, :],
                                start=(d == 0), stop=(d == DC - 1))
                        if i < 8:
                            nc.scalar.copy(vt[:, i, :, 0:HD], pvp[:])
                        else:
                            nc.vector.tensor_copy(vt[:, i, :, 0:HD], pvp[:])

                    def attn_gen(h, half, cpool):
                        """Generator: one yield per k-chunk j, one final
                        yield for the C eviction + reciprocal."""
                        pair, po = h // 2, 64 * (h % 2)
                        qb = HW_ * half
                        C = cpool.tile([HD + 1, HW_], f32, tag="C",
                                       name=f"C{h}{half}")
                        nj = 8 if half == 0 else NJ

                        def emit_pv(j, et_, qlo):
                            c0 = qlo
                            while c0 < qb + HW_:
                                c1 = min((c0 // 512 + 1) * 512, qb + HW_)
                                nc.tensor.matmul(
                                    C[:, c0 - qb:c1 - qb],
                                    vt[:, j, h, :],
                                    et_[:, c0 - qlo:c1 - qlo],
                                    start=(j == 0),
                                    stop=(j == 4 * (c0 // 512) + 3))
                                c0 = c1

                        pending = None
                        for j in range(nj):
                            qlo = max(qb, 128 * j)
                            w = qb + HW_ - qlo
                            kk = slice(128 * j, 128 * j + 128)
                            sc = scp.tile([128, HW_], f32, tag="sc",
                                          name=f"sc{h}{half}{j}")
                            for c0 in range(0, w, 512):
                                cw = min(512, w - c0)
                                nc.tensor.matmul(
                                    sc[:, c0:c0 + cw],
                                    kt[po:po + HD, pair, kk],
                                    qt[po:po + HD, pair,
                                       qlo + c0:qlo + c0 + cw],
                                    start=True, stop=True)
                            et_ = wrk.tile([128, HW_], f32r, tag="ex",
                                           bufs=3, name=f"ex{h}{half}{j}")
                            nc.scalar.activation(
                                et_[:, 0:w], sc[:, 0:w], AF.Exp, scale=0.125)
                            if qlo == 128 * j:   # diagonal: zero k > q
                                eng = nc.gpsimd if j % 2 == 0 else nc.vector
                                eng.tensor_tensor(
                                    et_[:, 0:128], et_[:, 0:128], msk[:],
                                    OP.mult)
                            if pending is not None:
                                emit_pv(*pending)
                            pending = (j, et_, qlo)
                            yield
                        emit_pv(*pending)
                        cs = slice(qb, qb + HW_)
                        idx = 2 * h + half
                        rp, rf = 32 * (idx % 4), idx // 4
                        if half == 0:   # ScalarE has slack in phase 1
                            nc.scalar.copy(cxt[po:po + HD, pair, cs],
                                           C[0:HD, :])
                        else:
                            nc.vector.tensor_copy(cxt[po:po + HD, pair, cs],
                                                  C[0:HD, :])
                        nc.vector.reciprocal(rcs[rp:rp + 1, rf, :],
                                             C[HD:HD + 1, :])
                        yield

                    def norm_unit(pair, half):
                        cs = slice(HW_ * half, HW_ * half + HW_)
                        rbs = wrk.tile([128, HW_], f32, tag="rbs", bufs=2,
                                       name=f"rbs{pair}{half}")
                        for sub in range(2):
                            idx = 2 * (2 * pair + sub) + half
                            rp, rf = 32 * (idx % 4), idx // 4
                            nc.sync.dma_start(
                                rbs[64 * sub:64 * sub + 64, :],
                                rcs[rp:rp + 1, rf, None, :]
                                .to_broadcast([1, 64, HW_]))
                        nc.vector.tensor_tensor(
                            cxt[:, pair, cs], cxt[:, pair, cs], rbs[:],
                            OP.mult)

                    def op_unit(i, osp):
                        op_ = osp.tile([128, D], f32, tag="op",
                                       name=f"op{i}")
                        ss = slice(128 * i, 128 * i + 128)
                        for pair in range(PAIRS):
                            for c0 in (0, 512):
                                cw = min(512, D - c0)
                                nc.tensor.matmul(
                                    op_[:, c0:c0 + cw],
                                    cxt[:, pair, ss],
                                    wot[:, pair, c0:c0 + cw],
                                    start=(pair == 0),
                                    stop=(pair == PAIRS - 1))
                        ot = wrk.tile([128, D], f32, tag="ot", bufs=4,
                                      name=f"ot{i}")
                        if i < 8 or i % 2 == 1:
                            nc.vector.tensor_copy(ot[:], op_[:])
                        else:
                            nc.scalar.copy(ot[:], op_[:])
                        (nc.sync if i % 2 == 0 else nc.scalar).dma_start(
                            o[ss, :], ot[:])

                    def step(g, n=1):
                        for _ in range(n):
                            next(g, None)

                    # ================= master weave =================
                    # Phase 1: all six heads' low halves (q < 1024), with
                    # pair-0 chunks 0-3, pair-1/2 chunks 0-1, V rows 0-1023.
                    qk_unit(0, wqt, qt, 0)
                    qk_unit(0, wkt, kt, 0)
                    qk_unit(0, wqt, qt, 1)
                    qk_unit(0, wkt, kt, 1)
                    pv_unit(0); pv_unit(1)
                    A = attn_gen(0, 0, cxp)
                    step(A, 2)
                    pv_unit(2); pv_unit(3)
                    step(A, 2)
                    pv_unit(4); pv_unit(5)
                    step(A, 2)
                    pv_unit(6); pv_unit(7)
                    step(A, 3)

                    A = attn_gen(1, 0, cxp)
                    step(A, 2); qk_unit(1, wqt, qt, 0)
                    step(A, 2); qk_unit(1, wkt, kt, 0)
                    step(A, 2); qk_unit(1, wqt, qt, 1)
                    step(A, 2); qk_unit(1, wkt, kt, 1)
                    step(A, 1)
                    norm_unit(0, 0)

                    A = attn_gen(2, 0, cxp)
                    step(A, 2); qk_unit(2, wqt, qt, 0)
                    step(A, 2); qk_unit(2, wkt, kt, 0)
                    step(A, 5)

                    A = attn_gen(3, 0, cxp)
                    step(A, 2); qk_unit(2, wqt, qt, 1)
                    step(A, 2); qk_unit(2, wkt, kt, 1)
                    step(A, 5)
                    norm_unit(1, 0)

                    A = attn_gen(4, 0, cxp)
                    step(A, 2); qk_unit(0, wqt, qt, 2)
                    step(A, 2); qk_unit(0, wkt, kt, 2)
                    step(A, 5)

                    A = attn_gen(5, 0, cxp)
                    step(A, 2); qk_unit(0, wqt, qt, 3)
                    step(A, 2); qk_unit(0, wkt, kt, 3)
                    step(A, 5)
                    norm_unit(2, 0)

                    # Phase 2: high halves, remaining projections, V rows
                    # 1024-2047, and the first half of the out-projection.
                    A = attn_gen(0, 1, cxp)
                    for i in range(8, 16):
                        step(A, 1)
                        pv_unit(i)
                    step(A, 9)

                    A = attn_gen(1, 1, cxp)
                    step(A, 2); qk_unit(1, wqt, qt, 2)
                    step(A, 2); qk_unit(1, wkt, kt, 2)
                    step(A, 2); qk_unit(1, wqt, qt, 3)
                    step(A, 2); qk_unit(1, wkt, kt, 3)
                    step(A, 9)
                    norm_unit(0, 1)

                    A = attn_gen(2, 1, cxp)
                    step(A, 2); qk_unit(2, wqt, qt, 2)
                    step(A, 2); qk_unit(2, wkt, kt, 2)
                    step(A, 2); qk_unit(2, wqt, qt, 3)
                    step(A, 2); qk_unit(2, wkt, kt, 3)
                    step(A, 9)

                    # projections done: recycle their PSUM banks for the
                    # out-projection of rows 0-1023
                    pps_cm.__exit__(None, None, None)
                    osp1_cm = tc.tile_pool(name="osp1", bufs=1, space="PSUM")
                    osp1 = osp1_cm.__enter__()

                    A = attn_gen(3, 1, cxp)
                    op_unit(0, osp1)
                    step(A, 3); op_unit(1, osp1)
                    step(A, 3); op_unit(2, osp1)
                    step(A, 3); op_unit(3, osp1)
                    step(A, 3); op_unit(4, osp1)
                    step(A, 5)
                    norm_unit(1, 1)

                    A = attn_gen(4, 1, cxp)
                    op_unit(5, osp1)
                    step(A, 4); op_unit(6, osp1)
                    step(A, 4); op_unit(7, osp1)
                    step(A, 9)

                    A = attn_gen(5, 1, cxp)
                    step(A, 17)
                    norm_unit(2, 1)

                    osp1_cm.__exit__(None, None, None)
                    cxp_cm.__exit__(None, None, None)
                    scp_cm.__exit__(None, None, None)

                    # ---- tail: out-projection of rows 1024-2047 ----
                    with tc.tile_pool(name="osp2", bufs=4,
                                      space="PSUM") as osp2:
                        for i in range(8, 16):
                            op_unit(i, osp2)

    nc.compile()
    return nc


def _get_program(reps=1):
    if reps not in _CACHE:
        _CACHE[reps] = _build_program(reps)
    return _CACHE[reps]


def _tf32_round(x):
    """Round-to-nearest-even to TF32 (10-bit mantissa), kept as float32 bits."""
    b = np.ascontiguousarray(x, np.float32).view(np.uint32)
    lsb = (b >> 13) & 1
    b = (b + np.uint32(0x0FFF) + lsb) & np.uint32(0xFFFFE000)
    return b.view(np.float32)


def _bf16(x):
    import ml_dtypes
    return np.ascontiguousarray(x, np.float32).astype(ml_dtypes.bfloat16)


def make_in_maps(embeds, Wq, Wk, Wv, Wo):
    cosF, sinM = _rope_tables()
    perm = _qk_perm()
    mask = (np.arange(128)[:, None] <= np.arange(128)[None, :]).astype(np.float32)
    eTs = [_bf16(embeds[b].T) for b in range(B)]
    cosF_b, sinM_b = _bf16(cosF), _bf16(sinM)
    in_maps = []
    for c in range(N_CORES):
        b, hg = c // 2, c % 2
        hs = slice(hg * 384, hg * 384 + 384)
        in_maps.append({
            "eT": eTs[b],
            "wq": _bf16(Wq[hs].T[:, perm]),
            "wk": _bf16(Wk[hs].T[:, perm]),
            "wv": _bf16(Wv[hs].T),
            "wo": _tf32_round(Wo[:, hs].T),
            "cosF": cosF_b, "sinM": sinM_b, "mask": mask,
        })
    return in_maps


def kernel(embeds, Wq, Wk, Wv, Wo, bo):
    from concourse.bass_utils import run_bass_kernel_spmd

    embeds = np.asarray(embeds, np.float32)
    Wq = np.asarray(Wq, np.float32)
    Wk = np.asarray(Wk, np.float32)
    Wv = np.asarray(Wv, np.float32)
    Wo = np.asarray(Wo, np.float32)
    bo = np.asarray(bo, np.float32)

    nc = _get_program()
    in_maps = make_in_maps(embeds, Wq, Wk, Wv, Wo)
    res = run_bass_kernel_spmd(nc, in_maps, list(range(N_CORES))).results
    out = np.empty((B, S, D), np.float32)
    for b in range(B):
        out[b] = res[2 * b]["o"] + res[2 * b + 1]["o"] + bo
    return out


# revision 21
# speedup vs baseline: 1.0818x; 1.0818x over previous
"""Trainium2 Bass kernel for causal multi-head attention with RoPE.

Problem: B=4, S=2048, D=768, H=12, HD=64 (torch-Linear style projections,
rotary embeddings on q/k, causal softmax, output projection + bias).

Sharding across 8 NeuronCores: core c handles batch c//2 and head-group
c%2 (6 of 12 heads). Each core computes a partial output projection
(its heads' contribution to ctx @ Wo.T); the host sums the two partials
per batch and adds the bias. No device collectives.

Per-core kernel:
  - Projections run in bf16 (inputs/weights shipped as bf16; PE rate is
    identical to fp32r and DMA bytes halve). Q^T/K^T [hd, S] computed via
    pre-transposed weights; RoPE fused into PSUM eviction: cos-mult on
    DVE, the four rotate-half sin-mults on Pool (32-partition slices),
    final add on DVE. V [S, hd] with an appended ones column (Pool evict).
  - Attention in fp32r: per head, scores^T [k, q] = K_j^T.T @ Q^T (causal
    chunks only), exp on ScalarE with scale=1/8 folded in, PV with [V|1]
    stationary so PSUM row 64 is the softmax denominator for free; PV is
    emitted one k-chunk behind exp so PE's in-order queue never stalls.
  - The whole program is a fine-grained weave: all six heads' low halves
    (q < 1024) run first, with projection chunks and V blocks inserted
    between attention k-steps so PE always has independent work while
    ScalarE catches up on exp; high halves follow the same pattern, and
    the out-projection of rows 0-1023 is folded into the last head's
    attention stream. Normalization multiplies whole head-pairs
    ([128,1024] per op) with reciprocals DMA-broadcast across partitions.
"""

import numpy as np

B, S, D, H = 4, 2048, 768, 12
HD = D // H          # 64
N_CORES = 8
HEADS_PER_CORE = 6
PAIRS = 3            # head pairs per core
DC = D // 128        # 6 contraction chunks
MC = HEADS_PER_CORE * HD // 128   # 3 output-dim chunks (pairs)
NJ = S // 128        # 16 k-chunks
HW_ = 1024           # q-half width

_CACHE = {}


def _rope_tables():
    """cos/sin tables in the pair-interleaved hd layout.

    q/k features are stored with rotate-half partners adjacent (partition
    2f holds x1[f], partition 2f+1 holds x2[f], per head) so the rotate
    becomes a swap of adjacent lanes inside each 32-lane quadrant — a
    single DVE stream_shuffle.  sinM carries the rotate signs: row 2f
    multiplies the shuffled x2[f] by -sin, row 2f+1 multiplies x1[f] by
    +sin."""
    inv_freq = 1.0 / (10000.0 ** (np.arange(0, HD, 2, dtype=np.float64) / HD))
    ang = np.arange(S, dtype=np.float64)[:, None] * inv_freq[None, :]  # [S, 32]
    cos = np.cos(ang).astype(np.float32)   # [S, 32]
    sin = np.sin(ang).astype(np.float32)
    cosF = np.empty((128, S), np.float32)
    sinM = np.empty((128, S), np.float32)
    for h2 in range(2):
        for f in range(32):
            p = 64 * h2 + 2 * f
            cosF[p] = cosF[p + 1] = cos.T[f]
            sinM[p] = -sin.T[f]
            sinM[p + 1] = sin.T[f]
    return cosF, sinM


def _qk_perm():
    """Column permutation of Wq/Wk into the pair-interleaved layout."""
    perm = np.empty(384, np.int64)
    for h6 in range(HEADS_PER_CORE):
        base = 64 * h6
        for f in range(32):
            perm[base + 2 * f] = base + f
            perm[base + 2 * f + 1] = base + 32 + f
    return perm


def _build_program(reps=1):
    import concourse.bacc as bacc
    import concourse.mybir as mybir
    import concourse.tile as tile

    f32 = mybir.dt.float32
    f32r = mybir.dt.float32r
    bf16 = mybir.dt.bfloat16
    AF = mybir.ActivationFunctionType
    OP = mybir.AluOpType

    SWAP_MASK = [i ^ 1 for i in range(32)]   # swap adjacent lanes

    nc = bacc.Bacc("TRN2", target_bir_lowering=False, debug=False,
                   num_devices=N_CORES)

    eT = nc.declare_dram_parameter("eT", [D, S], bf16, isOutput=False)
    wq = nc.declare_dram_parameter("wq", [D, 384], bf16, isOutput=False)
    wk = nc.declare_dram_parameter("wk", [D, 384], bf16, isOutput=False)
    wv = nc.declare_dram_parameter("wv", [D, 384], bf16, isOutput=False)
    wo = nc.declare_dram_parameter("wo", [384, D], f32r, isOutput=False)
    cosF_d = nc.declare_dram_parameter("cosF", [128, S], bf16, isOutput=False)
    sinM_d = nc.declare_dram_parameter("sinM", [128, S], bf16, isOutput=False)
    mask_d = nc.declare_dram_parameter("mask", [128, 128], f32, isOutput=False)
    o = nc.declare_dram_parameter("o", [S, D], f32, isOutput=True)

    with tile.TileContext(nc) as tc, \
            nc.allow_low_precision(reason="bf16 projections / tf32 attention"):
        with tc.tile_pool(name="const", bufs=1) as cp:
            cosF = cp.tile([128, S], bf16)
            sinM = cp.tile([128, S], bf16)
            msk = cp.tile([128, 128], f32)

            qt = cp.tile([128, PAIRS, S], f32r)
            kt = cp.tile([128, PAIRS, S], f32r)
            vt = cp.tile([128, NJ, HEADS_PER_CORE, HD + 1], f32r)
            nc.vector.memset(vt[:, :, :, HD].bitcast(mybir.dt.uint32),
                             0x3F800000)
            wot = cp.tile([128, MC, D], f32r)
            rcs = cp.tile([128, 3, HW_], f32)

            eT_r = eT[:].rearrange("(n p) s -> p n s", p=128)
            wq_r = wq[:].rearrange("(n p) m -> p n m", p=128)
            wk_r = wk[:].rearrange("(n p) m -> p n m", p=128)
            wv_r = wv[:].rearrange("(n p) m -> p n m", p=128)
            wo_r = wo[:].rearrange("(n p) m -> p n m", p=128)

            for _rep in range(reps):
                with (
                    tc.tile_pool(name="pc", bufs=1) as pc,
                    tc.tile_pool(name="wts", bufs=1) as wts,
                    tc.tile_pool(name="wrk", bufs=1) as wrk,
                ):
                    cxt = pc.tile([128, PAIRS, S], f32r)   # unnormalized ctx^T

                    # ---- DMA prologue ----
                    # SP queue: the eT stream (all four 512-col chunks live
                    # at once; bf16 keeps the ring at 12 KB/partition).
                    ets = []
                    for cc in range(4):
                        cols = slice(512 * cc, 512 * cc + 512)
                        etA = wrk.tile([128, 3, 512], bf16, tag="et", bufs=8,
                                       name=f"eA{cc}")
                        nc.sync.dma_start(etA[:], eT_r[:, 0:3, cols])
                        etB = wrk.tile([128, 3, 512], bf16, tag="et", bufs=8,
                                       name=f"eB{cc}")
                        nc.sync.dma_start(etB[:], eT_r[:, 3:6, cols])
                        ets.append((etA, etB))

                    # Act queue: weights + tables, ordered by first use.
                    wqt = wts.tile([128, DC, 384], bf16)
                    nc.scalar.dma_start(wqt[:], wq_r)
                    nc.scalar.dma_start(cosF[:, 0:512], cosF_d[:, 0:512])
                    nc.scalar.dma_start(sinM[:, 0:512], sinM_d[:, 0:512])
                    wkt = wts.tile([128, DC, 384], bf16)
                    nc.scalar.dma_start(wkt[:], wk_r)
                    nc.scalar.dma_start(msk[:], mask_d[:])
                    wvt = wts.tile([128, DC, 384], bf16)
                    nc.scalar.dma_start(wvt[:], wv_r)
                    nc.scalar.dma_start(cosF[:, 512:1024], cosF_d[:, 512:1024])
                    nc.scalar.dma_start(sinM[:, 512:1024], sinM_d[:, 512:1024])
                    nc.scalar.dma_start(wot[:], wo_r)
                    nc.scalar.dma_start(cosF[:, 1024:], cosF_d[:, 1024:])
                    nc.scalar.dma_start(sinM[:, 1024:], sinM_d[:, 1024:])

                    scp_cm = tc.tile_pool(name="scp", bufs=2, space="PSUM")
                    scp = scp_cm.__enter__()
                    cxp_cm = tc.tile_pool(name="cxp", bufs=1, space="PSUM")
                    cxp = cxp_cm.__enter__()
                    pps_cm = tc.tile_pool(name="pps", bufs=2, space="PSUM")
                    pps = pps_cm.__enter__()

                    def et(cc, d):
                        return ets[cc][0 if d < 3 else 1][:, d % 3, :]

                    def qk_unit(pair, wt, dst, cc):
                        """Q^T or K^T for one pair, one 512-col chunk, with
                        RoPE fused into the PSUM eviction."""
                        cols = slice(512 * cc, 512 * cc + 512)
                        tag = "q" if dst is qt else "k"
                        ps = pps.tile([128, 512], f32, tag="ps",
                                      name=f"ps{tag}{pair}{cc}")
                        for d in range(DC):
                            nc.tensor.matmul(
                                ps[:],
                                wt[:, d, 128 * pair:128 * pair + 128],
                                et(cc, d),
                                start=(d == 0), stop=(d == DC - 1))
                        psw = wrk.tile([128, 512], f32, tag="sh", bufs=2,
                                       name=f"w{tag}{pair}{cc}")
                        nc.vector.stream_shuffle(psw[:], ps[:], SWAP_MASK)
                        t_t = wrk.tile([128, 512], f32, tag="t", bufs=2,
                                       name=f"t{tag}{pair}{cc}")
                        nc.vector.tensor_tensor(
                            t_t[:], ps[:], cosF[:, cols], OP.mult)
                        u_t = wrk.tile([128, 512], f32, tag="u", bufs=2,
                                       name=f"u{tag}{pair}{cc}")
                        nc.vector.tensor_tensor(
                            u_t[:], psw[:], sinM[:, cols], OP.mult)
                        nc.vector.tensor_tensor(
                            dst[:, pair, cols], t_t[:], u_t[:], OP.add)

                    def pv_unit(i):
                        """V rows [128i, 128i+128) for all heads."""
                        cc, io = i // 4, 128 * (i % 4)
                        pvp = pps.tile([128, 384], f32, tag="ps",
                                       name=f"pv{i}")
                        for d in range(DC):
                            nc.tensor.matmul(
                                pvp[:], et(cc, d)[:, io:io + 128],
                                wvt[:, d, :],
                                start=(d == 0), stop=(d == DC - 1))
                        if i < 8:
                            nc.scalar.copy(vt[:, i, :, 0:HD], pvp[:])
                        else:
                            nc.vector.tensor_copy(vt[:, i, :, 0:HD], pvp[:])

                    def attn_gen(h, half, cpool):
                        """Generator: one yield per k-chunk j, one final
                        yield for the C eviction + reciprocal."""
                        pair, po = h // 2, 64 * (h % 2)
                        qb = HW_ * half
                        C = cpool.tile([HD + 1, HW_], f32, tag="C",
                                       name=f"C{h}{half}")
                        nj = 8 if half == 0 else NJ

                        def emit_pv(j, et_, qlo):
                            c0 = qlo
                            while c0 < qb + HW_:
                                c1 = min((c0 // 512 + 1) * 512, qb + HW_)
                                nc.tensor.matmul(
                                    C[:, c0 - qb:c1 - qb],
                                    vt[:, j, h, :],
                                    et_[:, c0 - qlo:c1 - qlo],
                                    start=(j == 0),
                                    stop=(j == 4 * (c0 // 512) + 3))
                                c0 = c1

                        pending = None
                        for j in range(nj):
                            qlo = max(qb, 128 * j)
                            w = qb + HW_ - qlo
                            kk = slice(128 * j, 128 * j + 128)
                            sc = scp.tile([128, HW_], f32, tag="sc",
                                          name=f"sc{h}{half}{j}")
                            for c0 in range(0, w, 512):
                                cw = min(512, w - c0)
                                nc.tensor.matmul(
                                    sc[:, c0:c0 + cw],
                                    kt[po:po + HD, pair, kk],
                                    qt[po:po + HD, pair,
                                       qlo + c0:qlo + c0 + cw],
                                    start=True, stop=True)
                            et_ = wrk.tile([128, HW_], f32r, tag="ex",
                                           bufs=3, name=f"ex{h}{half}{j}")
                            nc.scalar.activation(
                                et_[:, 0:w], sc[:, 0:w], AF.Exp, scale=0.125)
                            if qlo == 128 * j:   # diagonal: zero k > q
                                eng = nc.gpsimd if j % 2 == 0 else nc.vector
                                eng.tensor_tensor(
                                    et_[:, 0:128], et_[:, 0:128], msk[:],
                                    OP.mult)
                            if pending is not None:
                                emit_pv(*pending)
                            pending = (j, et_, qlo)
                            yield
                        emit_pv(*pending)
                        cs = slice(qb, qb + HW_)
                        idx = 2 * h + half
                        rp, rf = 32 * (idx % 4), idx // 4
                        if half == 0:   # ScalarE has slack in phase 1
                            nc.scalar.copy(cxt[po:po + HD, pair, cs],
                                           C[0:HD, :])
                        else:
                            nc.vector.tensor_copy(cxt[po:po + HD, pair, cs],
                                                  C[0:HD, :])
                        nc.vector.reciprocal(rcs[rp:rp + 1, rf, :],
                                             C[HD:HD + 1, :])
                        yield

                    def norm_unit(pair, half):
                        cs = slice(HW_ * half, HW_ * half + HW_)
                        rbs = wrk.tile([128, HW_], f32, tag="rbs", bufs=2,
                                       name=f"rbs{pair}{half}")
                        for sub in range(2):
                            idx = 2 * (2 * pair + sub) + half
                            rp, rf = 32 * (idx % 4), idx // 4
                            nc.sync.dma_start(
                                rbs[64 * sub:64 * sub + 64, :],
                                rcs[rp:rp + 1, rf, None, :]
                                .to_broadcast([1, 64, HW_]))
                        nc.vector.tensor_tensor(
                            cxt[:, pair, cs], cxt[:, pair, cs], rbs[:],
                            OP.mult)

                    def op_unit(i, osp):
                        op_ = osp.tile([128, D], f32, tag="op",
                                       name=f"op{i}")
                        ss = slice(128 * i, 128 * i + 128)
                        for pair in range(PAIRS):
                            for c0 in (0, 512):
                                cw = min(512, D - c0)
                                nc.tensor.matmul(
                                    op_[:, c0:c0 + cw],
                                    cxt[:, pair, ss],
                                    wot[:, pair, c0:c0 + cw],
                                    start=(pair == 0),
                                    stop=(pair == PAIRS - 1))
                        ot = wrk.tile([128, D], f32, tag="ot", bufs=4,
                                      name=f"ot{i}")
                        if i < 8 or i % 2 == 1:
                            nc.vector.tensor_copy(ot[:], op_[:])
                        else:
                            nc.scalar.copy(ot[:], op_[:])
                        (nc.sync if i % 2 == 0 else nc.scalar).dma_start(
                            o[ss, :], ot[:])

                    def step(g, n=1):
                        for _ in range(n):
                            next(g, None)

                    # ================= master weave =================
                    # Phase 1: all six heads' low halves (q < 1024), with
                    # pair-0 chunks 0-3, pair-1/2 chunks 0-1, V rows 0-1023.
                    qk_unit(0, wqt, qt, 0)
                    qk_unit(0, wkt, kt, 0)
                    qk_unit(0, wqt, qt, 1)
                    qk_unit(0, wkt, kt, 1)
                    pv_unit(0); pv_unit(1)
                    A = attn_gen(0, 0, cxp)
                    step(A, 2)
                    pv_unit(2); pv_unit(3)
                    step(A, 2)
                    pv_unit(4); pv_unit(5)
                    step(A, 2)
                    pv_unit(6); pv_unit(7)
                    step(A, 3)

                    A = attn_gen(1, 0, cxp)
                    step(A, 2); qk_unit(1, wqt, qt, 0)
                    step(A, 2); qk_unit(1, wkt, kt, 0)
                    step(A, 2); qk_unit(1, wqt, qt, 1)
                    step(A, 2); qk_unit(1, wkt, kt, 1)
                    step(A, 1)
                    norm_unit(0, 0)

                    A = attn_gen(2, 0, cxp)
                    step(A, 2); qk_unit(2, wqt, qt, 0)
                    step(A, 2); qk_unit(2, wkt, kt, 0)
                    step(A, 5)

                    A = attn_gen(3, 0, cxp)
                    step(A, 2); qk_unit(2, wqt, qt, 1)
                    step(A, 2); qk_unit(2, wkt, kt, 1)
                    step(A, 5)
                    norm_unit(1, 0)

                    A = attn_gen(4, 0, cxp)
                    step(A, 2); qk_unit(0, wqt, qt, 2)
                    step(A, 2); qk_unit(0, wkt, kt, 2)
                    step(A, 5)

                    A = attn_gen(5, 0, cxp)
                    step(A, 2); qk_unit(0, wqt, qt, 3)
                    step(A, 2); qk_unit(0, wkt, kt, 3)
                    step(A, 5)
                    norm_unit(2, 0)

                    # Phase 2: high halves, remaining projections, V rows
                    # 1024-2047, and the first half of the out-projection.
                    A = attn_gen(0, 1, cxp)
                    for i in range(8, 16):
                        step(A, 1)
                        pv_unit(i)
                    step(A, 9)

                    A = attn_gen(1, 1, cxp)
                    step(A, 2); qk_unit(1, wqt, qt, 2)
                    step(A, 2); qk_unit(1, wkt, kt, 2)
                    step(A, 2); qk_unit(1, wqt, qt, 3)
                    step(A, 2); qk_unit(1, wkt, kt, 3)
                    step(A, 9)
                    norm_unit(0, 1)

                    A = attn_gen(2, 1, cxp)
                    step(A, 2); qk_unit(2, wqt, qt, 2)
                    step(A, 2); qk_unit(2, wkt, kt, 2)
                    step(A, 2); qk_unit(2, wqt, qt, 3)
                    step(A, 2); qk_unit(2, wkt, kt, 3)
                    step(A, 9)

                    # projections done: recycle their PSUM banks for the
                    # out-projection of rows 0-1023
                    pps_cm.__exit__(None, None, None)
                    osp1_cm = tc.tile_pool(name="osp1", bufs=1, space="PSUM")
                    osp1 = osp1_cm.__enter__()

                    A = attn_gen(3, 1, cxp)
                    op_unit(0, osp1)
                    step(A, 3); op_unit(1, osp1)
                    step(A, 3); op_unit(2, osp1)
                    step(A, 3); op_unit(3, osp1)
                    step(A, 3); op_unit(4, osp1)
                    step(A, 5)
                    norm_unit(1, 1)

                    A = attn_gen(4, 1, cxp)
                    op_unit(5, osp1)
                    step(A, 4); op_unit(6, osp1)
                    step(A, 4); op_unit(7, osp1)
                    step(A, 9)

                    A = attn_gen(5, 1, cxp)
                    step(A, 17)
                    norm_unit(2, 1)

                    osp1_cm.__exit__(None, None, None)
                    cxp_cm.__exit__(None, None, None)
                    scp_cm.__exit__(None, None, None)

                    # ---- tail: out-projection of rows 1024-2047 ----
                    with tc.tile_pool(name="osp2", bufs=4,
                                      space="PSUM") as osp2:
                        for i in range(8, 16):
                            op_unit(i, osp2)

    nc.compile()
    return nc


def _get_program(reps=1):
    if reps not in _CACHE:
        _CACHE[reps] = _build_program(reps)
    return _CACHE[reps]


def _tf32_round(x):
    """Round-to-nearest-even to TF32 (10-bit mantissa), kept as float32 bits."""
    b = np.ascontiguousarray(x, np.float32).view(np.uint32)
    lsb = (b >> 13) & 1
    b = (b + np.uint32(0x0FFF) + lsb) & np.uint32(0xFFFFE000)
    return b.view(np.float32)


def _bf16(x):
    import ml_dtypes
    return np.ascontiguousarray(x, np.float32).astype(ml_dtypes.bfloat16)


def make_in_maps(embeds, Wq, Wk, Wv, Wo):
    cosF, sinM = _rope_tables()
    perm = _qk_perm()
    mask = (np.arange(128)[:, None] <= np.arange(128)[None, :]).astype(np.float32)
    eTs = [_bf16(embeds[b].T) for b in range(B)]
    cosF_b, sinM_b = _bf16(cosF), _bf16(sinM)
    in_maps = []
    for c in range(N_CORES):
        b, hg = c // 2, c % 2
        hs = slice(hg * 384, hg * 384 + 384)
        in_maps.append({
            "eT": eTs[b],
            "wq": _bf16(Wq[hs].T[:, perm]),
            "wk": _bf16(Wk[hs].T[:, perm]),
            "wv": _bf16(Wv[hs].T),
            "wo": _tf32_round(Wo[:, hs].T),
            "cosF": cosF_b, "sinM": sinM_b, "mask": mask,
        })
    return in_maps


def kernel(embeds, Wq, Wk, Wv, Wo, bo):
    from concourse.bass_utils import run_bass_kernel_spmd

    embeds = np.asarray(embeds, np.float32)
    Wq = np.asarray(Wq, np.float32)
    Wk = np.asarray(Wk, np.float32)
    Wv = np.asarray(Wv, np.float32)
    Wo = np.asarray(Wo, np.float32)
    bo = np.asarray(bo, np.float32)

    nc = _get_program()
    in_maps = make_in_maps(embeds, Wq, Wk, Wv, Wo)
    res = run_bass_kernel_spmd(nc, in_maps, list(range(N_CORES))).results
    out = np.empty((B, S, D), np.float32)
    for b in range(B):
        out[b] = res[2 * b]["o"] + res[2 * b + 1]["o"] + bo
    return out
